# revision 14
# baseline (speedup 1.0000x reference)
"""Trainium2 Bass kernel for CustomBertSelfAttention (no head split).

reference:
    q = hs @ Wq + bq; k = hs @ Wk + bk; v = hs @ Wv + bv        # [B,S,D]
    scores = (q @ k^T) / sqrt(64) + mask                         # [B,S,S]
    probs  = softmax(scores, -1)
    out    = probs @ v                                           # [B,S,D]

B=8, S=2048, D=1024.  Sharding: data-parallel over batch, one batch
element per NeuronCore (8 cores), no collectives.

Per-core plan (all matmuls in fp32r = TF32-like dtype, full PE rate):
  1a. hs -> hsT [d, s] via PE transpose mode (after ~3.5us of junk
      matmuls to warm the HAM clock gate), PSUM -> SBUF copies cast to
      fp32r on DVE.
  1b. projections with contraction d on partitions:
        kT[dout, s] (SBUF resident), qT[dout, s] -> DRAM spill,
        v[t, d] natural -> DRAM spill (spills on the gpsimd queue so the
        sync queue stays clear for phase-2 loads).
  2.  per s-block of 256 columns:
        scoresT[t, s] = sum_dk matmuls, kT chunks stationary (PSUM fp32)
        exp on ACT: exp(scores*0.125 + mask[t]) -> SBUF fp32r
        rowsum over t: ones-vector-stationary matmuls -> rowsumT [1, s],
        then per-128 PE transposes ([1,1] identity) + DVE reciprocal
        context[s, d] = sum_tc expT-chunk @ v-chunk (PSUM)
        normalize via tensor_scalar_mul on the PSUM->SBUF copy, DMA out.

Known DMA pitfalls baked in: 4-byte-scatter / broadcast constant loads
(mask/biases) are slow DIRECT2D patterns and sit behind all hs chunks in
the sync queue; the fp32r producer rule and the even-moving-dim rule for
fp32r matmuls are documented in the project memory.
"""

import sys

sys.path.insert(0, "/opt/trn_rl_repo")

from contextlib import ExitStack

import numpy as np

import concourse.bass as bass
import concourse.mybir as mybir
import concourse.tile as tile
from concourse import bacc
from concourse.bass_utils import run_bass_kernel_spmd
from concourse.masks import make_identity

B, S, D = 8, 2048, 1024
NCORES = 8
PD = 128            # partition dim
DK = D // PD        # 8 contraction chunks
SC = S // PD        # 16 sequence chunks
NT = 512            # matmul moving-dim tile (one PSUM bank of fp32)
SBLK = 256          # attention s-block
NBLK = S // SBLK
F32 = mybir.dt.float32
F32R = mybir.dt.float32r
EXP = mybir.ActivationFunctionType.Exp

_compiled_nc = None


def _build():
    nc = bacc.Bacc(
        "TRN2",
        target_bir_lowering=False,
        debug=False,
        num_devices=NCORES,
        enable_asserts=False,
    )
    hs = nc.dram_tensor("hidden_states", [S, D], F32, kind="ExternalInput").ap()
    mask = nc.dram_tensor("attention_mask", [1, S], F32, kind="ExternalInput").ap()
    Wq = nc.dram_tensor("Wq", [D, D], F32, kind="ExternalInput").ap()
    Wk = nc.dram_tensor("Wk", [D, D], F32, kind="ExternalInput").ap()
    Wv = nc.dram_tensor("Wv", [D, D], F32, kind="ExternalInput").ap()
    bq = nc.dram_tensor("bq", [D], F32, kind="ExternalInput").ap()
    bk = nc.dram_tensor("bk", [D], F32, kind="ExternalInput").ap()
    bv = nc.dram_tensor("bv", [D], F32, kind="ExternalInput").ap()
    out = nc.dram_tensor("context", [S, D], F32, kind="ExternalOutput").ap()

    with tile.TileContext(nc) as tc, ExitStack() as ctx:
        persist = ctx.enter_context(tc.tile_pool(name="persist", bufs=1))
        dramp = ctx.enter_context(tc.tile_pool(name="dram", bufs=1, space="DRAM"))
        qT_dram = dramp.tile([D, S], F32R)
        v_dram = dramp.tile([S, D], F32R)

        kT = persist.tile([PD, DK, S], F32R)

        # mask[t] laid out [p, tc] so bias slice [:, tc] is per-partition.
        # DMAs for these constants are emitted later (the hs loads must be
        # first in the sync queue; the bv broadcast alone is an 11.5us
        # DIRECT2D replication that would stall kernel start).
        mask_sb = persist.tile([PD, SC], F32)
        bq_sb = persist.tile([PD, DK], F32)
        bk_sb = persist.tile([PD, DK], F32)
        bv_row = persist.tile([PD, D], F32)

        ident = persist.tile([PD, PD], F32)
        make_identity(nc, ident)
        # fp32r matmuls need an even moving-dim count (2 results/cycle),
        # so the rowsum uses a [PD, 2] ones operand and a [PD, 2] psum.
        ones32 = persist.tile([PD, 2], F32)
        nc.vector.memset(ones32, 1.0)
        ones_r = persist.tile([PD, 2], F32R)
        nc.vector.tensor_copy(out=ones_r, in_=ones32)

        # attention-phase pools that must NOT reuse phase-1 SBUF/PSUM space:
        # allocated at the bottom of the stack so their first DMAs/matmuls
        # don't inherit WAR deps on the last phase-1 readers.
        qp = ctx.enter_context(tc.tile_pool(name="qsl", bufs=1))
        psc = ctx.enter_context(tc.tile_pool(name="psc", bufs=3, space="PSUM"))

        def load_q_slice(sb):
            q_sl = qp.tile([PD, DK, SBLK], F32R, name="q_sl", tag="q_sl")
            nc.sync.dma_start(
                out=q_sl,
                in_=qT_dram[:, sb * SBLK : (sb + 1) * SBLK].rearrange(
                    "(dk p) s -> p dk s", p=PD
                ),
            )
            return q_sl

        with ExitStack() as p1:
            hstp = p1.enter_context(tc.tile_pool(name="hsT_pool", bufs=1))
            # 4 column-tiles (one per 512-wide s-tile) so projections can
            # start as soon as their columns are transposed.
            hsT_st = [
                hstp.tile([PD, DK, NT], F32R, name=f"hsT{st}", tag=f"hsT{st}")
                for st in range(S // NT)
            ]

            def hsT(dk, lo, hi):
                st, off = lo // NT, lo % NT
                assert hi - lo <= NT and hi <= (st + 1) * NT
                return hsT_st[st][:, dk, off : off + (hi - lo)]

            # Wv is the one full-size weight load; issued after the first
            # couple of hs chunks (so the gpsimd Q7 clears the kernel
            # preamble sync first), still ~100us before the v projection.
            wvp = p1.enter_context(tc.tile_pool(name="wvp", bufs=1))
            wv = wvp.tile([PD, DK, D], F32R)

            # ---- phase 1a: hs -> hsT (PE fast-transpose mode)
            with (
                tc.tile_pool(name="hsload", bufs=4) as hsp,
                tc.tile_pool(name="ptr", bufs=4, space="PSUM") as ptr,
            ):
                # ~3.5us of junk fp32 matmuls while the first hs chunk loads:
                # transpose-mode doesn't count as PE-busy for the HAM clock
                # gate, so without this the whole transpose phase runs at the
                # cold 1.2 GHz rate.  DMA-out so DCE keeps it.
                warm_ps = ptr.tile([PD, PD], F32, name="warm_ps", tag="warm_ps", bufs=1)
                for _ in range(8):
                    nc.tensor.matmul(
                        out=warm_ps, lhsT=ident, rhs=ident, start=True, stop=True
                    )
                warm_sb = hsp.tile([PD, PD], F32, name="warm_sb", tag="warm_sb", bufs=1)
                nc.vector.tensor_copy(out=warm_sb, in_=warm_ps)
                warm_dram = dramp.tile([PD, PD], F32, name="warm_dram", tag="warm_dram")
                nc.sync.dma_start(out=warm_dram[:, :], in_=warm_sb)
                for sc in range(SC):
                    hchunk = hsp.tile([PD, D], F32)
                    nc.sync.dma_start(out=hchunk, in_=hs[sc * PD : (sc + 1) * PD, :])
                    if sc == 2:
                        nc.gpsimd.dma_start(
                            out=wv, in_=Wv.rearrange("(dk p) n -> p dk n", p=PD)
                        )
                    if sc == SC - 1:
                        # 4-byte-scatter / broadcast constant loads are slow
                        # DIRECT2D patterns (mask alone ~14us) - keep them
                        # behind all 16 hs chunks in the sync queue.
                        nc.sync.dma_start(
                            out=bk_sb, in_=bk.rearrange("(c p) -> p c", p=PD)
                        )
                        nc.sync.dma_start(
                            out=bq_sb, in_=bq.rearrange("(c p) -> p c", p=PD)
                        )
                        nc.sync.dma_start(
                            out=mask_sb,
                            in_=mask[0, :].rearrange("(c p) -> p c", p=PD),
                        )
                        bv_bcast = bass.AP(
                            tensor=bv.tensor, offset=bv.offset, ap=[[0, PD], *bv.ap]
                        )
                        nc.sync.dma_start(out=bv_row, in_=bv_bcast)
                    for dk in range(DK):
                        pst = ptr.tile([PD, PD], F32)
                        nc.tensor.transpose(
                            out=pst,
                            in_=hchunk[:, dk * PD : (dk + 1) * PD],
                            identity=ident,
                        )
                        nc.vector.tensor_copy(
                            out=hsT(dk, sc * PD, (sc + 1) * PD), in_=pst
                        )

            # ---- phase 1b: projections
            def project_qk(W, bias_sb, writer):
                with (
                    tc.tile_pool(name="wp", bufs=2) as wp,
                    tc.tile_pool(name="pp", bufs=3, space="PSUM") as pp,
                ):
                    for m in range(DK):
                        wm = wp.tile([PD, DK, PD], F32R)
                        nc.gpsimd.dma_start(
                            out=wm,
                            in_=W[:, m * PD : (m + 1) * PD].rearrange(
                                "(dk p) j -> p dk j", p=PD
                            ),
                        )
                        for st in range(S // NT):
                            ps = pp.tile([PD, NT], F32)
                            for dk in range(DK):
                                nc.tensor.matmul(
                                    out=ps,
                                    lhsT=wm[:, dk, :],
                                    rhs=hsT(dk, st * NT, (st + 1) * NT),
                                    start=(dk == 0),
                                    stop=(dk == DK - 1),
                                )
                            writer(m, st, ps)

            # k projection: straight into resident kT
            def k_writer(m, st, ps):
                nc.vector.tensor_scalar_add(
                    out=kT[:, m, st * NT : (st + 1) * NT],
                    in0=ps,
                    scalar1=bk_sb[:, m : m + 1],
                )

            project_qk(Wk, bk_sb, k_writer)

            # q projection: stage per (m, st) chunk, spill each to DRAM
            with tc.tile_pool(name="qstage", bufs=2) as qsp:

                def q_writer(m, st, ps):
                    qchunk = qsp.tile([PD, NT], F32R, name="qchunk", tag="qchunk")
                    nc.vector.tensor_scalar_add(
                        out=qchunk, in0=ps, scalar1=bq_sb[:, m : m + 1]
                    )
                    nc.sync.dma_start(
                        out=qT_dram[m * PD : (m + 1) * PD, st * NT : (st + 1) * NT],
                        in_=qchunk,
                    )

                project_qk(Wq, bq_sb, q_writer)

            # first attention q-slice: qp has its own reserved SBUF, so this
            # runs as soon as the q spills above complete
            q_next = load_q_slice(0)

            # v projection: natural [t, d] layout, spill to DRAM
            # (spills go via gpsimd/SWDGE so the sync queue stays clear for
            # the first attention q-slice + v reload)
            with (
                tc.tile_pool(name="pv", bufs=3, space="PSUM") as pv,
                tc.tile_pool(name="vstage", bufs=2) as vsp,
            ):
                for tcn in range(SC):
                    vstage = vsp.tile([PD, D], F32R)
                    for dt in range(D // NT):
                        ps = pv.tile([PD, NT], F32)
                        for dk in range(DK):
                            nc.tensor.matmul(
                                out=ps,
                                lhsT=hsT(dk, tcn * PD, (tcn + 1) * PD),
                                rhs=wv[:, dk, dt * NT : (dt + 1) * NT],
                                start=(dk == 0),
                                stop=(dk == DK - 1),
                            )
                        nc.vector.tensor_add(
                            out=vstage[:, dt * NT : (dt + 1) * NT],
                            in0=ps,
                            in1=bv_row[:, dt * NT : (dt + 1) * NT],
                        )
                    nc.gpsimd.dma_start(
                        out=v_dram[tcn * PD : (tcn + 1) * PD, :], in_=vstage
                    )

        # ---- phase 2: attention
        with (
            tc.tile_pool(name="vsb", bufs=1) as vp,
            tc.tile_pool(name="expp", bufs=2) as epool,
            tc.tile_pool(name="outp", bufs=2) as opool,
            tc.tile_pool(name="rcp", bufs=4) as rpool,
            tc.tile_pool(name="pctx", bufs=2, space="PSUM") as pctx,
            tc.tile_pool(name="prs", bufs=2, space="PSUM") as prs,
        ):
            v_sb = vp.tile([PD, SC, D], F32R)
            vr = v_dram.rearrange("(c p) d -> p c d", p=PD)
            for c4 in range(4):
                nc.sync.dma_start(
                    out=v_sb[:, c4 * 4 : (c4 + 1) * 4, :],
                    in_=vr[:, c4 * 4 : (c4 + 1) * 4, :],
                )
            for sb in range(NBLK):
                q_sl = q_next
                exp_sb = epool.tile([PD, SC, SBLK], F32R)
                for tcn in range(SC):
                    ps = psc.tile([PD, SBLK], F32)
                    for dk in range(DK):
                        nc.tensor.matmul(
                            out=ps,
                            lhsT=kT[:, dk, tcn * PD : (tcn + 1) * PD],
                            rhs=q_sl[:, dk, :],
                            start=(dk == 0),
                            stop=(dk == DK - 1),
                        )
                    nc.scalar.activation(
                        out=exp_sb[:, tcn, :],
                        in_=ps,
                        func=EXP,
                        scale=0.125,
                        bias=mask_sb[:, tcn : tcn + 1],
                    )
                if sb + 1 < NBLK:
                    q_next = load_q_slice(sb + 1)
                # rowsum over t: ones as the stationary operand (1-col LDW),
                # giving rowsumT [1, SBLK]; then per-128 transpose via PE
                # (identity [1,1]) to get per-partition [128,1] reciprocals.
                psr = prs.tile([1, SBLK], F32, bufs=1)
                for tcn in range(SC):
                    nc.tensor.matmul(
                        out=psr,
                        lhsT=ones_r[:, 0:1],
                        rhs=exp_sb[:, tcn, :],
                        start=(tcn == 0),
                        stop=(tcn == SC - 1),
                    )
                rs_sb = rpool.tile([1, SBLK], F32, name="rs_sb", tag="rs_sb")
                nc.vector.tensor_copy(out=rs_sb, in_=psr)
                recips = []
                for ss in range(SBLK // PD):
                    ptp = prs.tile([PD, 1], F32, name="ptp", tag="ptp", bufs=2)
                    nc.tensor.transpose(
                        out=ptp,
                        in_=rs_sb[0:1, ss * PD : (ss + 1) * PD],
                        identity=ident[0:1, 0:1],
                    )
                    recip_t = rpool.tile([PD, 1], F32, name="recip_t", tag="recip_t")
                    nc.vector.reciprocal(out=recip_t, in_=ptp)
                    recips.append(recip_t)
                for ss in range(SBLK // PD):
                    recip = recips[ss]
                    ostage = opool.tile([PD, D], F32)
                    for dt in range(D // NT):
                        pc = pctx.tile([PD, NT], F32)
                        for tcn in range(SC):
                            nc.tensor.matmul(
                                out=pc,
                                lhsT=exp_sb[:, tcn, ss * PD : (ss + 1) * PD],
                                rhs=v_sb[:, tcn, dt * NT : (dt + 1) * NT],
                                start=(tcn == 0),
                                stop=(tcn == SC - 1),
                            )
                        nc.vector.tensor_scalar_mul(
                            out=ostage[:, dt * NT : (dt + 1) * NT],
                            in0=pc,
                            scalar1=recip,
                        )
                    row = sb * SBLK + ss * PD
                    nc.sync.dma_start(out=out[row : row + PD, :], in_=ostage)

    nc.compile()
    return nc


def _get_compiled():
    global _compiled_nc
    if _compiled_nc is None:
        _compiled_nc = _build()
    return _compiled_nc


def _run(inputs, **kwargs):
    hs = np.asarray(inputs["hidden_states"], dtype=np.float32)
    mask = np.asarray(inputs["attention_mask"], dtype=np.float32)
    ws = {
        k: np.ascontiguousarray(np.asarray(inputs[k], dtype=np.float32))
        for k in ("Wq", "bq", "Wk", "bk", "Wv", "bv")
    }
    nc = _get_compiled()
    in_maps = [
        {
            "hidden_states": np.ascontiguousarray(hs[i]),
            "attention_mask": np.ascontiguousarray(mask[i]),
            **ws,
        }
        for i in range(NCORES)
    ]
    r = run_bass_kernel_spmd(nc, in_maps, list(range(NCORES)), **kwargs)
    out = np.stack([r.results[i]["context"] for i in range(NCORES)], axis=0)
    return out, r


def kernel(**inputs) -> np.ndarray:
    out, _ = _run(inputs)
    return out


if __name__ == "__main__":
    rng = np.random.default_rng(0)
    scale = 1.0 / np.sqrt(D)
    inputs = {
        "hidden_states": rng.standard_normal((B, S, D)).astype(np.float32),
        "attention_mask": np.zeros((B, 1, S), np.float32),
        "Wq": (rng.standard_normal((D, D)) * scale).astype(np.float32),
        "bq": np.zeros(D, np.float32),
        "Wk": (rng.standard_normal((D, D)) * scale).astype(np.float32),
        "bk": np.zeros(D, np.float32),
        "Wv": (rng.standard_normal((D, D)) * scale).astype(np.float32),
        "bv": np.zeros(D, np.float32),
    }
    got = kernel(**inputs)

    hs64 = inputs["hidden_states"].astype(np.float64)
    q = hs64 @ inputs["Wq"].astype(np.float64)
    k = hs64 @ inputs["Wk"].astype(np.float64)
    v = hs64 @ inputs["Wv"].astype(np.float64)
    sc = np.einsum("bsd,btd->bst", q, k) / 8.0
    sc -= sc.max(axis=-1, keepdims=True)
    p = np.exp(sc)
    p /= p.sum(axis=-1, keepdims=True)
    ref = np.einsum("bst,btd->bsd", p, v)
    err = np.abs(got.astype(np.float64) - ref)
    print(
        f"absmax={err.max():.3e} rel_vs_scale={err.max() / np.abs(ref).max():.3e} "
        f"rms_rel={np.sqrt((err**2).mean()) / np.sqrt((ref**2).mean()):.3e}"
    )


# revision 15
# speedup vs baseline: 1.0490x; 1.0490x over previous
"""Trainium2 Bass kernel for CustomBertSelfAttention (no head split).

reference:
    q = hs @ Wq + bq; k = hs @ Wk + bk; v = hs @ Wv + bv        # [B,S,D]
    scores = (q @ k^T) / sqrt(64) + mask                         # [B,S,S]
    probs  = softmax(scores, -1)
    out    = probs @ v                                           # [B,S,D]

B=8, S=2048, D=1024.  Sharding: data-parallel over batch, one batch
element per NeuronCore (8 cores), no collectives.

Per-core plan (all matmuls in fp32r = TF32-like dtype, full PE rate):
  1a. hs -> hsT [d, s] via PE transpose mode (after ~3.5us of junk
      matmuls to warm the HAM clock gate), PSUM -> SBUF copies cast to
      fp32r on DVE.
  1b. projections with contraction d on partitions:
        kT[dout, s] (SBUF resident), qT[dout, s] -> DRAM spill,
        v[t, d] natural -> DRAM spill (spills on the gpsimd queue so the
        sync queue stays clear for phase-2 loads).
  2.  per s-block of 256 columns:
        scoresT[t, s] = sum_dk matmuls, kT chunks stationary (PSUM fp32)
        exp on ACT: exp(scores*0.125 + mask[t]) -> SBUF fp32r
        rowsum over t: ones-vector-stationary matmuls -> rowsumT [1, s],
        then per-128 PE transposes ([1,1] identity) + DVE reciprocal
        context[s, d] = sum_tc expT-chunk @ v-chunk (PSUM)
        normalize via tensor_scalar_mul on the PSUM->SBUF copy, DMA out.

Known DMA pitfalls baked in: 4-byte-scatter / broadcast constant loads
(mask/biases) are slow DIRECT2D patterns and sit behind all hs chunks in
the sync queue; the fp32r producer rule and the even-moving-dim rule for
fp32r matmuls are documented in the project memory.
"""

import sys

sys.path.insert(0, "/opt/trn_rl_repo")

from contextlib import ExitStack

import numpy as np

import concourse.bass as bass
import concourse.mybir as mybir
import concourse.tile as tile
from concourse import bacc
from concourse.bass_utils import run_bass_kernel_spmd
from concourse.masks import make_identity

B, S, D = 8, 2048, 1024
NCORES = 8
PD = 128            # partition dim
DK = D // PD        # 8 contraction chunks
SC = S // PD        # 16 sequence chunks
NT = 512            # matmul moving-dim tile (one PSUM bank of fp32)
SBLK = 256          # attention s-block
NBLK = S // SBLK
F32 = mybir.dt.float32
F32R = mybir.dt.float32r
EXP = mybir.ActivationFunctionType.Exp

_compiled_nc = None


def _build():
    nc = bacc.Bacc(
        "TRN2",
        target_bir_lowering=False,
        debug=False,
        num_devices=NCORES,
        enable_asserts=False,
    )
    hs = nc.dram_tensor("hidden_states", [S, D], F32, kind="ExternalInput").ap()
    mask = nc.dram_tensor("attention_mask", [1, S], F32, kind="ExternalInput").ap()
    Wq = nc.dram_tensor("Wq", [D, D], F32, kind="ExternalInput").ap()
    Wk = nc.dram_tensor("Wk", [D, D], F32, kind="ExternalInput").ap()
    Wv = nc.dram_tensor("Wv", [D, D], F32, kind="ExternalInput").ap()
    bq = nc.dram_tensor("bq", [D], F32, kind="ExternalInput").ap()
    bk = nc.dram_tensor("bk", [D], F32, kind="ExternalInput").ap()
    bv = nc.dram_tensor("bv", [D], F32, kind="ExternalInput").ap()
    out = nc.dram_tensor("context", [S, D], F32, kind="ExternalOutput").ap()

    with tile.TileContext(nc) as tc, ExitStack() as ctx:
        persist = ctx.enter_context(tc.tile_pool(name="persist", bufs=1))
        dramp = ctx.enter_context(tc.tile_pool(name="dram", bufs=1, space="DRAM"))
        qT_dram = dramp.tile([D, S], F32R)
        v_dram = dramp.tile([S, D], F32R)

        kT = persist.tile([PD, DK, S], F32R)

        # mask[t] laid out [p, tc] so bias slice [:, tc] is per-partition.
        # DMAs for these constants are emitted later (the hs loads must be
        # first in the sync queue; the bv broadcast alone is an 11.5us
        # DIRECT2D replication that would stall kernel start).
        mask_sb = persist.tile([PD, SC], F32)
        bq_sb = persist.tile([PD, DK], F32)
        bk_sb = persist.tile([PD, DK], F32)
        bv_row = persist.tile([PD, D], F32)

        ident = persist.tile([PD, PD], F32)
        make_identity(nc, ident)
        # fp32r matmuls need an even moving-dim count (2 results/cycle),
        # so the rowsum uses a [PD, 2] ones operand and a [PD, 2] psum.
        ones32 = persist.tile([PD, 2], F32)
        nc.vector.memset(ones32, 1.0)
        ones_r = persist.tile([PD, 2], F32R)
        nc.vector.tensor_copy(out=ones_r, in_=ones32)

        with ExitStack() as p1:
            hstp = p1.enter_context(tc.tile_pool(name="hsT_pool", bufs=1))
            # 4 column-tiles (one per 512-wide s-tile) so projections can
            # start as soon as their columns are transposed.
            hsT_st = [
                hstp.tile([PD, DK, NT], F32R, name=f"hsT{st}", tag=f"hsT{st}")
                for st in range(S // NT)
            ]

            def hsT(dk, lo, hi):
                st, off = lo // NT, lo % NT
                assert hi - lo <= NT and hi <= (st + 1) * NT
                return hsT_st[st][:, dk, off : off + (hi - lo)]

            # Wv is the one full-size weight load; issued after the first
            # couple of hs chunks (so the gpsimd Q7 clears the kernel
            # preamble sync first), still ~100us before the v projection.
            wvp = p1.enter_context(tc.tile_pool(name="wvp", bufs=1))
            wv = wvp.tile([PD, DK, D], F32R)

            # ---- phase 1a: hs -> hsT (PE fast-transpose mode)
            with (
                tc.tile_pool(name="hsload", bufs=5) as hsp,
                tc.tile_pool(name="ptr", bufs=4, space="PSUM") as ptr,
            ):
                # ~3.5us of junk fp32 matmuls while the first hs chunk loads:
                # transpose-mode doesn't count as PE-busy for the HAM clock
                # gate, so without this the whole transpose phase runs at the
                # cold 1.2 GHz rate.  DMA-out so DCE keeps it.
                warm_ps = ptr.tile([PD, PD], F32, name="warm_ps", tag="warm_ps", bufs=1)
                for _ in range(8):
                    nc.tensor.matmul(
                        out=warm_ps, lhsT=ident, rhs=ident, start=True, stop=True
                    )
                warm_sb = hsp.tile([PD, PD], F32, name="warm_sb", tag="warm_sb", bufs=1)
                nc.vector.tensor_copy(out=warm_sb, in_=warm_ps)
                warm_dram = dramp.tile([PD, PD], F32, name="warm_dram", tag="warm_dram")
                nc.sync.dma_start(out=warm_dram[:, :], in_=warm_sb)
                for sc in range(SC):
                    hchunk = hsp.tile([PD, D], F32)
                    nc.sync.dma_start(out=hchunk, in_=hs[sc * PD : (sc + 1) * PD, :])
                    if sc == 2:
                        nc.gpsimd.dma_start(
                            out=wv, in_=Wv.rearrange("(dk p) n -> p dk n", p=PD)
                        )
                    if sc == SC - 1:
                        # 4-byte-scatter / broadcast constant loads are slow
                        # DIRECT2D patterns (mask alone ~14us) - keep them
                        # behind all 16 hs chunks in the sync queue.
                        nc.sync.dma_start(
                            out=bk_sb, in_=bk.rearrange("(c p) -> p c", p=PD)
                        )
                        nc.sync.dma_start(
                            out=bq_sb, in_=bq.rearrange("(c p) -> p c", p=PD)
                        )
                        nc.sync.dma_start(
                            out=mask_sb,
                            in_=mask[0, :].rearrange("(c p) -> p c", p=PD),
                        )
                        bv_bcast = bass.AP(
                            tensor=bv.tensor, offset=bv.offset, ap=[[0, PD], *bv.ap]
                        )
                        nc.sync.dma_start(out=bv_row, in_=bv_bcast)
                    for dk in range(DK):
                        pst = ptr.tile([PD, PD], F32)
                        nc.tensor.transpose(
                            out=pst,
                            in_=hchunk[:, dk * PD : (dk + 1) * PD],
                            identity=ident,
                        )
                        nc.vector.tensor_copy(
                            out=hsT(dk, sc * PD, (sc + 1) * PD), in_=pst
                        )

            # ---- phase 1b: projections
            def project_qk(W, bias_sb, writer):
                with (
                    tc.tile_pool(name="wp", bufs=3) as wp,
                    tc.tile_pool(name="pp", bufs=3, space="PSUM") as pp,
                ):
                    for m in range(DK):
                        wm = wp.tile([PD, DK, PD], F32R)
                        nc.gpsimd.dma_start(
                            out=wm,
                            in_=W[:, m * PD : (m + 1) * PD].rearrange(
                                "(dk p) j -> p dk j", p=PD
                            ),
                        )
                        for st in range(S // NT):
                            ps = pp.tile([PD, NT], F32)
                            for dk in range(DK):
                                nc.tensor.matmul(
                                    out=ps,
                                    lhsT=wm[:, dk, :],
                                    rhs=hsT(dk, st * NT, (st + 1) * NT),
                                    start=(dk == 0),
                                    stop=(dk == DK - 1),
                                )
                            writer(m, st, ps)

            # k projection: straight into resident kT
            def k_writer(m, st, ps):
                nc.vector.tensor_scalar_add(
                    out=kT[:, m, st * NT : (st + 1) * NT],
                    in0=ps,
                    scalar1=bk_sb[:, m : m + 1],
                )

            project_qk(Wk, bk_sb, k_writer)

            # q projection: stage per m-chunk, spill to DRAM
            with tc.tile_pool(name="qstage", bufs=2) as qsp:
                qstages = {}

                def q_writer(m, st, ps):
                    if st == 0:
                        qstages[m] = qsp.tile(
                            [PD, S], F32R, name="qstage_t", tag="qstage_t"
                        )
                    nc.vector.tensor_scalar_add(
                        out=qstages[m][:, st * NT : (st + 1) * NT],
                        in0=ps,
                        scalar1=bq_sb[:, m : m + 1],
                    )
                    if st == S // NT - 1:
                        nc.sync.dma_start(
                            out=qT_dram[m * PD : (m + 1) * PD, :], in_=qstages[m]
                        )

                project_qk(Wq, bq_sb, q_writer)

            # v projection: natural [t, d] layout, spill to DRAM
            # (spills go via gpsimd/SWDGE so the sync queue stays clear for
            # the first attention q-slice + v reload)
            with (
                tc.tile_pool(name="pv", bufs=3, space="PSUM") as pv,
                tc.tile_pool(name="vstage", bufs=2) as vsp,
            ):
                for tcn in range(SC):
                    vstage = vsp.tile([PD, D], F32R)
                    for dt in range(D // NT):
                        ps = pv.tile([PD, NT], F32)
                        for dk in range(DK):
                            nc.tensor.matmul(
                                out=ps,
                                lhsT=hsT(dk, tcn * PD, (tcn + 1) * PD),
                                rhs=wv[:, dk, dt * NT : (dt + 1) * NT],
                                start=(dk == 0),
                                stop=(dk == DK - 1),
                            )
                        nc.vector.tensor_add(
                            out=vstage[:, dt * NT : (dt + 1) * NT],
                            in0=ps,
                            in1=bv_row[:, dt * NT : (dt + 1) * NT],
                        )
                    nc.gpsimd.dma_start(
                        out=v_dram[tcn * PD : (tcn + 1) * PD, :], in_=vstage
                    )

        # ---- phase 2: attention
        with (
            tc.tile_pool(name="vsb", bufs=1) as vp,
            tc.tile_pool(name="qsl", bufs=2) as qp,
            tc.tile_pool(name="expp", bufs=2) as epool,
            tc.tile_pool(name="outp", bufs=2) as opool,
            tc.tile_pool(name="rcp", bufs=4) as rpool,
            tc.tile_pool(name="psc", bufs=3, space="PSUM") as psc,
            tc.tile_pool(name="pctx", bufs=2, space="PSUM") as pctx,
            tc.tile_pool(name="prs", bufs=2, space="PSUM") as prs,
        ):
            def load_q_slice(sb):
                q_sl = qp.tile([PD, DK, SBLK], F32R, name="q_sl", tag="q_sl")
                nc.sync.dma_start(
                    out=q_sl,
                    in_=qT_dram[:, sb * SBLK : (sb + 1) * SBLK].rearrange(
                        "(dk p) s -> p dk s", p=PD
                    ),
                )
                return q_sl

            q_next = load_q_slice(0)
            v_sb = vp.tile([PD, SC, D], F32R)
            vr = v_dram.rearrange("(c p) d -> p c d", p=PD)
            for c4 in range(4):
                nc.sync.dma_start(
                    out=v_sb[:, c4 * 4 : (c4 + 1) * 4, :],
                    in_=vr[:, c4 * 4 : (c4 + 1) * 4, :],
                )
            for sb in range(NBLK):
                q_sl = q_next
                exp_sb = epool.tile([PD, SC, SBLK], F32R)
                for tcn in range(SC):
                    ps = psc.tile([PD, SBLK], F32)
                    for dk in range(DK):
                        nc.tensor.matmul(
                            out=ps,
                            lhsT=kT[:, dk, tcn * PD : (tcn + 1) * PD],
                            rhs=q_sl[:, dk, :],
                            start=(dk == 0),
                            stop=(dk == DK - 1),
                        )
                    nc.scalar.activation(
                        out=exp_sb[:, tcn, :],
                        in_=ps,
                        func=EXP,
                        scale=0.125,
                        bias=mask_sb[:, tcn : tcn + 1],
                    )
                if sb + 1 < NBLK:
                    q_next = load_q_slice(sb + 1)
                # rowsum over t: ones as the stationary operand (1-col LDW),
                # giving rowsumT [1, SBLK]; then per-128 transpose via PE
                # (identity [1,1]) to get per-partition [128,1] reciprocals.
                psr = prs.tile([1, SBLK], F32, bufs=1)
                for tcn in range(SC):
                    nc.tensor.matmul(
                        out=psr,
                        lhsT=ones_r[:, 0:1],
                        rhs=exp_sb[:, tcn, :],
                        start=(tcn == 0),
                        stop=(tcn == SC - 1),
                    )
                rs_sb = rpool.tile([1, SBLK], F32, name="rs_sb", tag="rs_sb")
                nc.vector.tensor_copy(out=rs_sb, in_=psr)
                recips = []
                for ss in range(SBLK // PD):
                    ptp = prs.tile([PD, 1], F32, name="ptp", tag="ptp", bufs=2)
                    nc.tensor.transpose(
                        out=ptp,
                        in_=rs_sb[0:1, ss * PD : (ss + 1) * PD],
                        identity=ident[0:1, 0:1],
                    )
                    recip_t = rpool.tile([PD, 1], F32, name="recip_t", tag="recip_t")
                    nc.vector.reciprocal(out=recip_t, in_=ptp)
                    recips.append(recip_t)
                for ss in range(SBLK // PD):
                    recip = recips[ss]
                    ostage = opool.tile([PD, D], F32)
                    for dt in range(D // NT):
                        pc = pctx.tile([PD, NT], F32)
                        for tcn in range(SC):
                            nc.tensor.matmul(
                                out=pc,
                                lhsT=exp_sb[:, tcn, ss * PD : (ss + 1) * PD],
                                rhs=v_sb[:, tcn, dt * NT : (dt + 1) * NT],
                                start=(tcn == 0),
                                stop=(tcn == SC - 1),
                            )
                        nc.vector.tensor_scalar_mul(
                            out=ostage[:, dt * NT : (dt + 1) * NT],
                            in0=pc,
                            scalar1=recip,
                        )
                    row = sb * SBLK + ss * PD
                    nc.sync.dma_start(out=out[row : row + PD, :], in_=ostage)

    nc.compile()
    return nc


def _get_compiled():
    global _compiled_nc
    if _compiled_nc is None:
        _compiled_nc = _build()
    return _compiled_nc


def _run(inputs, **kwargs):
    hs = np.asarray(inputs["hidden_states"], dtype=np.float32)
    mask = np.asarray(inputs["attention_mask"], dtype=np.float32)
    ws = {
        k: np.ascontiguousarray(np.asarray(inputs[k], dtype=np.float32))
        for k in ("Wq", "bq", "Wk", "bk", "Wv", "bv")
    }
    nc = _get_compiled()
    in_maps = [
        {
            "hidden_states": np.ascontiguousarray(hs[i]),
            "attention_mask": np.ascontiguousarray(mask[i]),
            **ws,
        }
        for i in range(NCORES)
    ]
    r = run_bass_kernel_spmd(nc, in_maps, list(range(NCORES)), **kwargs)
    out = np.stack([r.results[i]["context"] for i in range(NCORES)], axis=0)
    return out, r


def kernel(**inputs) -> np.ndarray:
    out, _ = _run(inputs)
    return out


if __name__ == "__main__":
    rng = np.random.default_rng(0)
    scale = 1.0 / np.sqrt(D)
    inputs = {
        "hidden_states": rng.standard_normal((B, S, D)).astype(np.float32),
        "attention_mask": np.zeros((B, 1, S), np.float32),
        "Wq": (rng.standard_normal((D, D)) * scale).astype(np.float32),
        "bq": np.zeros(D, np.float32),
        "Wk": (rng.standard_normal((D, D)) * scale).astype(np.float32),
        "bk": np.zeros(D, np.float32),
        "Wv": (rng.standard_normal((D, D)) * scale).astype(np.float32),
        "bv": np.zeros(D, np.float32),
    }
    got = kernel(**inputs)

    hs64 = inputs["hidden_states"].astype(np.float64)
    q = hs64 @ inputs["Wq"].astype(np.float64)
    k = hs64 @ inputs["Wk"].astype(np.float64)
    v = hs64 @ inputs["Wv"].astype(np.float64)
    sc = np.einsum("bsd,btd->bst", q, k) / 8.0
    sc -= sc.max(axis=-1, keepdims=True)
    p = np.exp(sc)
    p /= p.sum(axis=-1, keepdims=True)
    ref = np.einsum("bst,btd->bsd", p, v)
    err = np.abs(got.astype(np.float64) - ref)
    print(
        f"absmax={err.max():.3e} rel_vs_scale={err.max() / np.abs(ref).max():.3e} "
        f"rms_rel={np.sqrt((err**2).mean()) / np.sqrt((ref**2).mean()):.3e}"
    )


# revision 16
# speedup vs baseline: 1.0538x; 1.0046x over previous
"""Trainium2 Bass kernel for CustomBertSelfAttention (no head split).

reference:
    q = hs @ Wq + bq; k = hs @ Wk + bk; v = hs @ Wv + bv        # [B,S,D]
    scores = (q @ k^T) / sqrt(64) + mask                         # [B,S,S]
    probs  = softmax(scores, -1)
    out    = probs @ v                                           # [B,S,D]

B=8, S=2048, D=1024.  Sharding: data-parallel over batch, one batch
element per NeuronCore (8 cores), no collectives.

Per-core plan (all matmuls in fp32r = TF32-like dtype, full PE rate):
  1a. hs -> hsT [d, s] via PE transpose mode (after ~3.5us of junk
      matmuls to warm the HAM clock gate), PSUM -> SBUF copies cast to
      fp32r on DVE.
  1b. projections with contraction d on partitions:
        kT[dout, s] (SBUF resident), qT[dout, s] -> DRAM spill,
        v[t, d] natural -> DRAM spill (spills on the gpsimd queue so the
        sync queue stays clear for phase-2 loads).
  2.  per s-block of 256 columns:
        scoresT[t, s] = sum_dk matmuls, kT chunks stationary (PSUM fp32)
        exp on ACT: exp(scores*0.125 + mask[t]) -> SBUF fp32r
        rowsum over t: ones-vector-stationary matmuls -> rowsumT [1, s],
        then per-128 PE transposes ([1,1] identity) + DVE reciprocal
        context[s, d] = sum_tc expT-chunk @ v-chunk (PSUM)
        normalize via tensor_scalar_mul on the PSUM->SBUF copy, DMA out.

Known DMA pitfalls baked in: 4-byte-scatter / broadcast constant loads
(mask/biases) are slow DIRECT2D patterns and sit behind all hs chunks in
the sync queue; the fp32r producer rule and the even-moving-dim rule for
fp32r matmuls are documented in the project memory.
"""

import sys

sys.path.insert(0, "/opt/trn_rl_repo")

from contextlib import ExitStack

import numpy as np

import concourse.bass as bass
import concourse.mybir as mybir
import concourse.tile as tile
from concourse import bacc
from concourse.bass_utils import run_bass_kernel_spmd
from concourse.masks import make_identity

B, S, D = 8, 2048, 1024
NCORES = 8
PD = 128            # partition dim
DK = D // PD        # 8 contraction chunks
SC = S // PD        # 16 sequence chunks
NT = 512            # matmul moving-dim tile (one PSUM bank of fp32)
SBLK = 256          # attention s-block
NBLK = S // SBLK
F32 = mybir.dt.float32
F32R = mybir.dt.float32r
EXP = mybir.ActivationFunctionType.Exp

_compiled_nc = None


def _build():
    nc = bacc.Bacc(
        "TRN2",
        target_bir_lowering=False,
        debug=False,
        num_devices=NCORES,
        enable_asserts=False,
    )
    hs = nc.dram_tensor("hidden_states", [S, D], F32, kind="ExternalInput").ap()
    mask = nc.dram_tensor("attention_mask", [1, S], F32, kind="ExternalInput").ap()
    Wq = nc.dram_tensor("Wq", [D, D], F32, kind="ExternalInput").ap()
    Wk = nc.dram_tensor("Wk", [D, D], F32, kind="ExternalInput").ap()
    Wv = nc.dram_tensor("Wv", [D, D], F32, kind="ExternalInput").ap()
    bq = nc.dram_tensor("bq", [D], F32, kind="ExternalInput").ap()
    bk = nc.dram_tensor("bk", [D], F32, kind="ExternalInput").ap()
    bv = nc.dram_tensor("bv", [D], F32, kind="ExternalInput").ap()
    out = nc.dram_tensor("context", [S, D], F32, kind="ExternalOutput").ap()

    with tile.TileContext(nc) as tc, ExitStack() as ctx:
        persist = ctx.enter_context(tc.tile_pool(name="persist", bufs=1))
        dramp = ctx.enter_context(tc.tile_pool(name="dram", bufs=1, space="DRAM"))
        qT_dram = dramp.tile([D, S], F32R)
        v_dram = dramp.tile([S, D], F32R)

        kT = persist.tile([PD, DK, S], F32R)

        # mask[t] laid out [p, tc] so bias slice [:, tc] is per-partition.
        # DMAs for these constants are emitted later (the hs loads must be
        # first in the sync queue; the bv broadcast alone is an 11.5us
        # DIRECT2D replication that would stall kernel start).
        mask_sb = persist.tile([PD, SC], F32)
        bq_sb = persist.tile([PD, DK], F32)
        bk_sb = persist.tile([PD, DK], F32)
        bv_row = persist.tile([PD, D], F32)

        ident = persist.tile([PD, PD], F32)
        make_identity(nc, ident)
        # fp32r matmuls need an even moving-dim count (2 results/cycle),
        # so the rowsum uses a [PD, 2] ones operand and a [PD, 2] psum.
        ones32 = persist.tile([PD, 2], F32)
        nc.vector.memset(ones32, 1.0)
        ones_r = persist.tile([PD, 2], F32R)
        nc.vector.tensor_copy(out=ones_r, in_=ones32)

        with ExitStack() as p1:
            hstp = p1.enter_context(tc.tile_pool(name="hsT_pool", bufs=1))
            # 4 column-tiles (one per 512-wide s-tile) so projections can
            # start as soon as their columns are transposed.
            hsT_st = [
                hstp.tile([PD, DK, NT], F32R, name=f"hsT{st}", tag=f"hsT{st}")
                for st in range(S // NT)
            ]

            def hsT(dk, lo, hi):
                st, off = lo // NT, lo % NT
                assert hi - lo <= NT and hi <= (st + 1) * NT
                return hsT_st[st][:, dk, off : off + (hi - lo)]

            # Wv is the one full-size weight load; issued after the first
            # couple of hs chunks (so the gpsimd Q7 clears the kernel
            # preamble sync first), still ~100us before the v projection.
            wvp = p1.enter_context(tc.tile_pool(name="wvp", bufs=1))
            wv = wvp.tile([PD, DK, D], F32R)

            # ---- phase 1a: hs -> hsT (PE fast-transpose mode)
            with (
                tc.tile_pool(name="hsload", bufs=6) as hsp,
                tc.tile_pool(name="ptr", bufs=6, space="PSUM") as ptr,
            ):
                # ~3.5us of junk fp32 matmuls while the first hs chunk loads:
                # transpose-mode doesn't count as PE-busy for the HAM clock
                # gate, so without this the whole transpose phase runs at the
                # cold 1.2 GHz rate.  DMA-out so DCE keeps it.
                warm_ps = ptr.tile([PD, PD], F32, name="warm_ps", tag="warm_ps", bufs=1)
                for _ in range(8):
                    nc.tensor.matmul(
                        out=warm_ps, lhsT=ident, rhs=ident, start=True, stop=True
                    )
                warm_sb = hsp.tile([PD, PD], F32, name="warm_sb", tag="warm_sb", bufs=1)
                nc.vector.tensor_copy(out=warm_sb, in_=warm_ps)
                warm_dram = dramp.tile([PD, PD], F32, name="warm_dram", tag="warm_dram")
                nc.sync.dma_start(out=warm_dram[:, :], in_=warm_sb)
                for sc in range(SC):
                    hchunk = hsp.tile([PD, D], F32)
                    nc.sync.dma_start(out=hchunk, in_=hs[sc * PD : (sc + 1) * PD, :])
                    if sc == 2:
                        nc.gpsimd.dma_start(
                            out=wv, in_=Wv.rearrange("(dk p) n -> p dk n", p=PD)
                        )
                    if sc == SC - 1:
                        # 4-byte-scatter / broadcast constant loads are slow
                        # DIRECT2D patterns (mask alone ~14us) - keep them
                        # behind all 16 hs chunks in the sync queue.
                        nc.sync.dma_start(
                            out=bk_sb, in_=bk.rearrange("(c p) -> p c", p=PD)
                        )
                        nc.sync.dma_start(
                            out=bq_sb, in_=bq.rearrange("(c p) -> p c", p=PD)
                        )
                        nc.sync.dma_start(
                            out=mask_sb,
                            in_=mask[0, :].rearrange("(c p) -> p c", p=PD),
                        )
                        bv_bcast = bass.AP(
                            tensor=bv.tensor, offset=bv.offset, ap=[[0, PD], *bv.ap]
                        )
                        nc.sync.dma_start(out=bv_row, in_=bv_bcast)
                    for dk in range(DK):
                        pst = ptr.tile([PD, PD], F32)
                        nc.tensor.transpose(
                            out=pst,
                            in_=hchunk[:, dk * PD : (dk + 1) * PD],
                            identity=ident,
                        )
                        nc.vector.tensor_copy(
                            out=hsT(dk, sc * PD, (sc + 1) * PD), in_=pst
                        )

            # ---- phase 1b: projections
            def project_qk(W, bias_sb, writer):
                with (
                    tc.tile_pool(name="wp", bufs=3) as wp,
                    tc.tile_pool(name="pp", bufs=3, space="PSUM") as pp,
                ):
                    for m in range(DK):
                        wm = wp.tile([PD, DK, PD], F32R)
                        nc.gpsimd.dma_start(
                            out=wm,
                            in_=W[:, m * PD : (m + 1) * PD].rearrange(
                                "(dk p) j -> p dk j", p=PD
                            ),
                        )
                        for st in range(S // NT):
                            ps = pp.tile([PD, NT], F32)
                            for dk in range(DK):
                                nc.tensor.matmul(
                                    out=ps,
                                    lhsT=wm[:, dk, :],
                                    rhs=hsT(dk, st * NT, (st + 1) * NT),
                                    start=(dk == 0),
                                    stop=(dk == DK - 1),
                                )
                            writer(m, st, ps)

            # k projection: straight into resident kT
            def k_writer(m, st, ps):
                nc.vector.tensor_scalar_add(
                    out=kT[:, m, st * NT : (st + 1) * NT],
                    in0=ps,
                    scalar1=bk_sb[:, m : m + 1],
                )

            project_qk(Wk, bk_sb, k_writer)

            # q projection: stage per m-chunk, spill to DRAM
            with tc.tile_pool(name="qstage", bufs=2) as qsp:
                qstages = {}

                def q_writer(m, st, ps):
                    if st == 0:
                        qstages[m] = qsp.tile(
                            [PD, S], F32R, name="qstage_t", tag="qstage_t"
                        )
                    nc.vector.tensor_scalar_add(
                        out=qstages[m][:, st * NT : (st + 1) * NT],
                        in0=ps,
                        scalar1=bq_sb[:, m : m + 1],
                    )
                    if st == S // NT - 1:
                        nc.sync.dma_start(
                            out=qT_dram[m * PD : (m + 1) * PD, :], in_=qstages[m]
                        )

                project_qk(Wq, bq_sb, q_writer)

            # v projection: natural [t, d] layout, spill to DRAM
            # (spills go via gpsimd/SWDGE so the sync queue stays clear for
            # the first attention q-slice + v reload)
            with (
                tc.tile_pool(name="pv", bufs=3, space="PSUM") as pv,
                tc.tile_pool(name="vstage", bufs=2) as vsp,
            ):
                for tcn in range(SC):
                    vstage = vsp.tile([PD, D], F32R)
                    for dt in range(D // NT):
                        ps = pv.tile([PD, NT], F32)
                        for dk in range(DK):
                            nc.tensor.matmul(
                                out=ps,
                                lhsT=hsT(dk, tcn * PD, (tcn + 1) * PD),
                                rhs=wv[:, dk, dt * NT : (dt + 1) * NT],
                                start=(dk == 0),
                                stop=(dk == DK - 1),
                            )
                        nc.vector.tensor_add(
                            out=vstage[:, dt * NT : (dt + 1) * NT],
                            in0=ps,
                            in1=bv_row[:, dt * NT : (dt + 1) * NT],
                        )
                    nc.gpsimd.dma_start(
                        out=v_dram[tcn * PD : (tcn + 1) * PD, :], in_=vstage
                    )

        # ---- phase 2: attention
        with (
            tc.tile_pool(name="vsb", bufs=1) as vp,
            tc.tile_pool(name="qsl", bufs=2) as qp,
            tc.tile_pool(name="expp", bufs=2) as epool,
            tc.tile_pool(name="outp", bufs=2) as opool,
            tc.tile_pool(name="rcp", bufs=4) as rpool,
            tc.tile_pool(name="psc", bufs=3, space="PSUM") as psc,
            tc.tile_pool(name="pctx", bufs=2, space="PSUM") as pctx,
            tc.tile_pool(name="prs", bufs=2, space="PSUM") as prs,
        ):
            def load_q_slice(sb):
                q_sl = qp.tile([PD, DK, SBLK], F32R, name="q_sl", tag="q_sl")
                nc.sync.dma_start(
                    out=q_sl,
                    in_=qT_dram[:, sb * SBLK : (sb + 1) * SBLK].rearrange(
                        "(dk p) s -> p dk s", p=PD
                    ),
                )
                return q_sl

            q_next = load_q_slice(0)
            v_sb = vp.tile([PD, SC, D], F32R)
            vr = v_dram.rearrange("(c p) d -> p c d", p=PD)
            for c4 in range(4):
                nc.sync.dma_start(
                    out=v_sb[:, c4 * 4 : (c4 + 1) * 4, :],
                    in_=vr[:, c4 * 4 : (c4 + 1) * 4, :],
                )
            for sb in range(NBLK):
                q_sl = q_next
                exp_sb = epool.tile([PD, SC, SBLK], F32R)
                for tcn in range(SC):
                    ps = psc.tile([PD, SBLK], F32)
                    for dk in range(DK):
                        nc.tensor.matmul(
                            out=ps,
                            lhsT=kT[:, dk, tcn * PD : (tcn + 1) * PD],
                            rhs=q_sl[:, dk, :],
                            start=(dk == 0),
                            stop=(dk == DK - 1),
                        )
                    nc.scalar.activation(
                        out=exp_sb[:, tcn, :],
                        in_=ps,
                        func=EXP,
                        scale=0.125,
                        bias=mask_sb[:, tcn : tcn + 1],
                    )
                if sb + 1 < NBLK:
                    q_next = load_q_slice(sb + 1)
                # rowsum over t: ones as the stationary operand (1-col LDW),
                # giving rowsumT [1, SBLK]; then per-128 transpose via PE
                # (identity [1,1]) to get per-partition [128,1] reciprocals.
                psr = prs.tile([1, SBLK], F32, bufs=1)
                for tcn in range(SC):
                    nc.tensor.matmul(
                        out=psr,
                        lhsT=ones_r[:, 0:1],
                        rhs=exp_sb[:, tcn, :],
                        start=(tcn == 0),
                        stop=(tcn == SC - 1),
                    )
                rs_sb = rpool.tile([1, SBLK], F32, name="rs_sb", tag="rs_sb")
                nc.vector.tensor_copy(out=rs_sb, in_=psr)
                recips = []
                for ss in range(SBLK // PD):
                    ptp = prs.tile([PD, 1], F32, name="ptp", tag="ptp", bufs=2)
                    nc.tensor.transpose(
                        out=ptp,
                        in_=rs_sb[0:1, ss * PD : (ss + 1) * PD],
                        identity=ident[0:1, 0:1],
                    )
                    recip_t = rpool.tile([PD, 1], F32, name="recip_t", tag="recip_t")
                    nc.vector.reciprocal(out=recip_t, in_=ptp)
                    recips.append(recip_t)
                for ss in range(SBLK // PD):
                    recip = recips[ss]
                    ostage = opool.tile([PD, D], F32)
                    for dt in range(D // NT):
                        pc = pctx.tile([PD, NT], F32)
                        for tcn in range(SC):
                            nc.tensor.matmul(
                                out=pc,
                                lhsT=exp_sb[:, tcn, ss * PD : (ss + 1) * PD],
                                rhs=v_sb[:, tcn, dt * NT : (dt + 1) * NT],
                                start=(tcn == 0),
                                stop=(tcn == SC - 1),
                            )
                        nc.vector.tensor_scalar_mul(
                            out=ostage[:, dt * NT : (dt + 1) * NT],
                            in0=pc,
                            scalar1=recip,
                        )
                    row = sb * SBLK + ss * PD
                    nc.sync.dma_start(out=out[row : row + PD, :], in_=ostage)

    nc.compile()
    return nc


def _get_compiled():
    global _compiled_nc
    if _compiled_nc is None:
        _compiled_nc = _build()
    return _compiled_nc


def _run(inputs, **kwargs):
    hs = np.asarray(inputs["hidden_states"], dtype=np.float32)
    mask = np.asarray(inputs["attention_mask"], dtype=np.float32)
    ws = {
        k: np.ascontiguousarray(np.asarray(inputs[k], dtype=np.float32))
        for k in ("Wq", "bq", "Wk", "bk", "Wv", "bv")
    }
    nc = _get_compiled()
    in_maps = [
        {
            "hidden_states": np.ascontiguousarray(hs[i]),
            "attention_mask": np.ascontiguousarray(mask[i]),
            **ws,
        }
        for i in range(NCORES)
    ]
    r = run_bass_kernel_spmd(nc, in_maps, list(range(NCORES)), **kwargs)
    out = np.stack([r.results[i]["context"] for i in range(NCORES)], axis=0)
    return out, r


def kernel(**inputs) -> np.ndarray:
    out, _ = _run(inputs)
    return out


if __name__ == "__main__":
    rng = np.random.default_rng(0)
    scale = 1.0 / np.sqrt(D)
    inputs = {
        "hidden_states": rng.standard_normal((B, S, D)).astype(np.float32),
        "attention_mask": np.zeros((B, 1, S), np.float32),
        "Wq": (rng.standard_normal((D, D)) * scale).astype(np.float32),
        "bq": np.zeros(D, np.float32),
        "Wk": (rng.standard_normal((D, D)) * scale).astype(np.float32),
        "bk": np.zeros(D, np.float32),
        "Wv": (rng.standard_normal((D, D)) * scale).astype(np.float32),
        "bv": np.zeros(D, np.float32),
    }
    got = kernel(**inputs)

    hs64 = inputs["hidden_states"].astype(np.float64)
    q = hs64 @ inputs["Wq"].astype(np.float64)
    k = hs64 @ inputs["Wk"].astype(np.float64)
    v = hs64 @ inputs["Wv"].astype(np.float64)
    sc = np.einsum("bsd,btd->bst", q, k) / 8.0
    sc -= sc.max(axis=-1, keepdims=True)
    p = np.exp(sc)
    p /= p.sum(axis=-1, keepdims=True)
    ref = np.einsum("bst,btd->bsd", p, v)
    err = np.abs(got.astype(np.float64) - ref)
    print(
        f"absmax={err.max():.3e} rel_vs_scale={err.max() / np.abs(ref).max():.3e} "
        f"rms_rel={np.sqrt((err**2).mean()) / np.sqrt((ref**2).mean()):.3e}"
    )
